# revision 21
# baseline (speedup 1.0000x reference)
"""HGCN embedding kernel for Trainium2 (8 NeuronCores, SPMD data-parallel).

Math: with the block-diagonal dense incidence produced by the reference
setup (every batch's 32 nodes on all 8 hyperedges), B_inv = 1/32,
D_inv = 1/8, and the propagation collapses to
    out[b, a] = mean_a'( input[b, a'] @ lin_w )          (same for all a)
so the whole module is
    y[b] = relu( mean_a(input[b,a,:]) @ (lin_w @ out_w) + hgcn_bias @ out_w + out_b )
    output[b, a, :] = y[b]

Sharding strategy: data-parallel over batch (512 batches/core).  Each core
streams its 16 MiB input shard in variable-size chunks (small first chunks
so the DVE reduction starts early, small last chunks so the post-stream
tail is short), tree-reduces the 32 agents on DVE, transposes the
per-group mean via PE, applies the folded weight matmul, ReLU, and writes
one row per batch ([512, 128] per core).  The host unshard step
concatenates the 8 shards and replicates each batch row across its 32
(identical) agent rows.

Tile-count hygiene: the TileContext exit emits an all-engine barrier round
per allocated tile/pool (~140 ns each, measured ~9 us for ~48 of them), so
all buffers are single tiles sliced manually, in two pools.
"""

import sys

import numpy as np

sys.path.insert(0, "/opt/trn_rl_repo")

BATCH = 4096
N_AG = 32
N_HE = 8
F_IN = 256
F_OUT = 128
NCORES = 8
BC = BATCH // NCORES          # 512 batches per core
GB = 128                      # batches per group (= SBUF partitions)
GROUPS = BC // GB             # 4

# agents per input chunk, per group: first chunks small (early DVE start),
# last chunks small (short post-stream tail)
CHUNK_PLAN = [
    [4, 4, 8, 16],
    [16, 16],
    [16, 16],
    [8, 8, 8, 4, 2, 2],
]
XT_SLOTS = 7                  # in-flight chunk buffers (DMA issue k+SLOTS is
                              # gated on DVE finishing chunk k — keep deep)
ALT_DMA = False               # alternating HWDGE queues measured ~7us SLOWER

HOST_BCAST = True             # device emits [BC, F_OUT]; host replicates x32

_NC_CACHE = {}
TRACE = False
LAST_RESULT = None


def _build_bass(with_bias):
    import concourse.bacc as bacc
    import concourse.mybir as mybir
    import concourse.tile as tile
    from concourse.masks import make_identity

    f32 = mybir.dt.float32
    bf16 = mybir.dt.bfloat16
    # the general with-bias variant stays all-f32; the (actual) zero-bias
    # path runs the reduction tree / transpose / matmul internals in bf16
    # (host-simulated rms rel err 4.3e-3 vs the 2e-2 gate)
    mdt = f32 if with_bias else bf16
    nc = bacc.Bacc("TRN2", target_bir_lowering=False, debug=False,
                   num_devices=NCORES)

    x = nc.declare_dram_parameter("x", [BC, N_AG, F_IN], f32, isOutput=False)
    w2 = nc.declare_dram_parameter("w2", [2, 128, F_OUT], mdt, isOutput=False)
    cvec = nc.declare_dram_parameter("cvec", [1, F_OUT], f32, isOutput=False)
    ones1 = nc.declare_dram_parameter("ones1", [1, 128], f32, isOutput=False)
    out = nc.declare_dram_parameter("out", [BC, F_OUT], f32, isOutput=True)

    xap = x.ap()
    outap = out.ap()
    relu = mybir.ActivationFunctionType.Relu

    # f32 working tiles are 2x the size; use fewer slots to stay in SBUF
    nslots = XT_SLOTS if mdt == bf16 else 5

    with tile.TileContext(nc) as tc:
        with (
            tc.tile_pool(name="sb", bufs=1) as sb,
            tc.tile_pool(name="ps", bufs=1, space="PSUM") as ps,
        ):
            w2t = sb.tile([128, 2, F_OUT], mdt)
            nc.scalar.dma_start(out=w2t[:], in_=w2.ap().rearrange("c p j -> p c j"))
            ident = sb.tile([128, 128], mdt)
            make_identity(nc, ident[:])
            if with_bias:
                ct = sb.tile([1, F_OUT], f32)
                nc.scalar.dma_start(out=ct[:], in_=cvec[:])
                o1 = sb.tile([1, 128], f32)
                nc.scalar.dma_start(out=o1[:], in_=ones1[:])

            xt = sb.tile([128, nslots, 16 * F_IN], f32)
            xb = sb.tile([128, nslots, 8 * F_IN], mdt)
            # dedicated buffers for the final small chunks: their DMA issues
            # must not wait on buffer recycling (the recycle wait rides a DVE
            # progress clock that lags the stream)
            ndedic = len(CHUNK_PLAN[-1]) - 3
            dedic = [(GROUPS - 1, c) for c in range(ndedic, ndedic + 3)]
            dag = [CHUNK_PLAN[-1][c] for _, c in dedic]
            xt3 = [sb.tile([128, ag * F_IN], f32, name=f"xt3_{i}")
                   for i, ag in enumerate(dag)]
            xb3 = [sb.tile([128, ag * F_IN // 2], mdt, name=f"xb3_{i}")
                   for i, ag in enumerate(dag)]
            ms = sb.tile([128, GROUPS, F_IN], mdt)
            mt = sb.tile([128, GROUPS, 2, GB], mdt)
            yt = sb.tile([128, GROUPS, F_OUT], f32)
            pt = ps.tile([128, 2, GB], mdt)
            py = ps.tile([128, F_OUT], f32)

            slot = 0
            dmac = 0
            for g in range(GROUPS):
                for c, ag in enumerate(CHUNK_PLAN[g]):
                    a0 = sum(CHUNK_PLAN[g][:c])
                    cols = ag * F_IN
                    if (g, c) in dedic:
                        di = dedic.index((g, c))
                        xtv, xbv = xt3[di][:], xb3[di][:]
                    else:
                        xtv = xt[:, slot, 0:cols]
                        xbv = xb[:, slot, 0:cols // 2]
                        slot = (slot + 1) % nslots
                    ieng = nc.scalar if (ALT_DMA and dmac % 2) else nc.sync
                    dmac += 1
                    ieng.dma_start(
                        out=xtv,
                        in_=xap[g * GB:(g + 1) * GB, a0:a0 + ag]
                        .rearrange("b a f -> b (a f)"))
                    # tree-reduce the chunk down to [128, 256]; the first
                    # chunk's last level writes ms directly, later chunks
                    # merge into ms
                    S = cols // 2
                    dst = (ms[:, g, :] if (c == 0 and S == F_IN)
                           else xbv[:, 0:S])
                    nc.vector.tensor_add(            # level 1: f32 -> mdt
                        dst, xtv[:, 0:S], xtv[:, S:cols])
                    while S > F_IN:
                        h = S // 2
                        dst = (ms[:, g, :] if (c == 0 and h == F_IN)
                               else xbv[:, 0:h])
                        nc.vector.tensor_add(
                            dst, xbv[:, 0:h], xbv[:, h:S])
                        S = h
                    if c > 0:
                        nc.vector.tensor_add(
                            ms[:, g, :], ms[:, g, :], xbv[:, 0:F_IN])

                for fc in range(2):
                    nc.tensor.transpose(
                        pt[:, fc, :], ms[:, g, fc * 128:(fc + 1) * 128],
                        ident[:])
                nc.scalar.copy(mt[:, g, :, :], pt[:, :, :])
                for fc in range(2):
                    nc.tensor.matmul(py[:], mt[:, g, fc, :], w2t[:, fc, :],
                                     start=(fc == 0),
                                     stop=(fc == 1 and not with_bias))
                if with_bias:
                    nc.tensor.matmul(py[:], o1[:], ct[:], start=False,
                                     stop=True)
                nc.scalar.activation(yt[:, g, :], py[:], relu)
                nc.scalar.dma_start(out=outap[g * GB:(g + 1) * GB],
                                    in_=yt[:, g, :])
    nc.compile()
    return nc


def _get_nc(with_bias):
    key = ("bias", with_bias)
    if key not in _NC_CACHE:
        _NC_CACHE[key] = _build_bass(with_bias)
    return _NC_CACHE[key]


def _is_block_pattern(node_idx, edge_idx):
    n = BATCH * N_AG * N_HE
    if node_idx.shape != (n,) or edge_idx.shape != (n,):
        return False
    i = np.arange(n, dtype=np.int64)
    if not np.array_equal(node_idx.astype(np.int64), i // N_HE):
        return False
    return np.array_equal(edge_idx.astype(np.int64),
                          (i // (N_AG * N_HE)) * N_HE + (i % N_HE))


def _fallback(inp, lin_w, hgcn_bias, out_w, out_b, node_idx, edge_idx):
    # general (host) path for arbitrary incidence — only used if the indices
    # are not the block-diagonal pattern produced by the reference setup
    n_nodes = BATCH * N_AG
    n_edges = BATCH * N_HE
    x = inp.reshape(-1, F_IN) @ lin_w
    node_idx = node_idx.astype(np.int64)
    edge_idx = edge_idx.astype(np.int64)
    D = np.bincount(node_idx, minlength=n_nodes).astype(np.float32)
    deg = np.bincount(edge_idx, minlength=n_edges).astype(np.float32)
    D_inv = np.where(D > 0, 1.0 / np.maximum(D, 1), 0.0).astype(np.float32)
    B_inv = np.where(deg > 0, 1.0 / np.maximum(deg, 1), 0.0).astype(np.float32)
    edge_feat = np.zeros((n_edges, F_OUT), np.float32)
    np.add.at(edge_feat, edge_idx, x[node_idx] * B_inv[edge_idx][:, None])
    outp = np.zeros((n_nodes, F_OUT), np.float32)
    np.add.at(outp, node_idx, edge_feat[edge_idx] * D_inv[node_idx][:, None])
    outp += hgcn_bias
    return np.maximum(outp @ out_w + out_b, 0.0)


def kernel(**inputs):
    global LAST_RESULT
    inp = np.ascontiguousarray(np.asarray(inputs["input"], np.float32))
    lin_w = np.asarray(inputs["lin_w"], np.float32)
    hgcn_bias = np.asarray(inputs["hgcn_bias"], np.float32)
    out_w = np.asarray(inputs["out_w"], np.float32)
    out_b = np.asarray(inputs["out_b"], np.float32)
    node_idx = np.asarray(inputs["node_idx"])
    edge_idx = np.asarray(inputs["edge_idx"])

    if not _is_block_pattern(node_idx, edge_idx):
        return _fallback(inp, lin_w, hgcn_bias, out_w, out_b,
                         node_idx, edge_idx)

    # fold: y = relu(mean_a(input) @ (lin_w @ out_w) + hgcn_bias @ out_w + out_b)
    w64 = lin_w.astype(np.float64) @ out_w.astype(np.float64)
    W = (w64 / N_AG).astype(np.float32)
    c = (hgcn_bias.astype(np.float64) @ out_w.astype(np.float64)
         + out_b).astype(np.float32)
    with_bias = bool(np.any(c != 0.0))

    w2 = np.ascontiguousarray(W.reshape(2, 128, F_OUT))
    if not with_bias:
        import ml_dtypes
        w2 = np.ascontiguousarray(w2.astype(ml_dtypes.bfloat16))
    cvec = np.ascontiguousarray(c.reshape(1, F_OUT))
    ones1 = np.ones((1, 128), np.float32)

    from concourse.bass_utils import run_bass_kernel_spmd

    nc = _get_nc(with_bias)
    in_maps = [
        {"x": inp[i * BC:(i + 1) * BC], "w2": w2, "cvec": cvec,
         "ones1": ones1}
        for i in range(NCORES)
    ]
    res = run_bass_kernel_spmd(nc, in_maps, list(range(NCORES)), trace=TRACE)
    LAST_RESULT = res
    y = np.concatenate([res.results[i]["out"] for i in range(NCORES)],
                       axis=0)                          # [BATCH, F_OUT]
    full = np.broadcast_to(y[:, None, :], (BATCH, N_AG, F_OUT))
    return np.ascontiguousarray(full).reshape(BATCH * N_AG, F_OUT)


# revision 22
# speedup vs baseline: 1.0095x; 1.0095x over previous
"""HGCN embedding kernel for Trainium2 (8 NeuronCores, SPMD data-parallel).

Math: with the block-diagonal dense incidence produced by the reference
setup (every batch's 32 nodes on all 8 hyperedges), B_inv = 1/32,
D_inv = 1/8, and the propagation collapses to
    out[b, a] = mean_a'( input[b, a'] @ lin_w )          (same for all a)
so the whole module is
    y[b] = relu( mean_a(input[b,a,:]) @ (lin_w @ out_w) + hgcn_bias @ out_w + out_b )
    output[b, a, :] = y[b]

Sharding strategy: data-parallel over batch (512 batches/core).  Each core
streams its 16 MiB input shard in variable-size chunks (small first chunks
so the DVE reduction starts early, small last chunks so the post-stream
tail is short), tree-reduces the 32 agents on DVE, transposes the
per-group mean via PE, applies the folded weight matmul, ReLU, and writes
one row per batch ([512, 128] per core).  The host unshard step
concatenates the 8 shards and replicates each batch row across its 32
(identical) agent rows.

Tile-count hygiene: the TileContext exit emits an all-engine barrier round
per allocated tile/pool (~140 ns each, measured ~9 us for ~48 of them), so
all buffers are single tiles sliced manually, in two pools.
"""

import sys

import numpy as np

sys.path.insert(0, "/opt/trn_rl_repo")

BATCH = 4096
N_AG = 32
N_HE = 8
F_IN = 256
F_OUT = 128
NCORES = 8
BC = BATCH // NCORES          # 512 batches per core
GB = 128                      # batches per group (= SBUF partitions)
GROUPS = BC // GB             # 4

# agents per input chunk, per group: first chunks small (early DVE start),
# last chunks small (short post-stream tail)
CHUNK_PLAN = [
    [16, 16],
    [16, 16],
    [16, 16],
    [16, 8, 4, 2, 2],
]
XT_SLOTS = 7                  # in-flight chunk buffers (DMA issue k+SLOTS is
                              # gated on DVE finishing chunk k — keep deep)
ALT_DMA = False               # alternating HWDGE queues measured ~7us SLOWER

HOST_BCAST = True             # device emits [BC, F_OUT]; host replicates x32

_NC_CACHE = {}
TRACE = False
LAST_RESULT = None


def _build_bass(with_bias):
    import concourse.bacc as bacc
    import concourse.mybir as mybir
    import concourse.tile as tile
    from concourse.masks import make_identity

    f32 = mybir.dt.float32
    bf16 = mybir.dt.bfloat16
    # the general with-bias variant stays all-f32; the (actual) zero-bias
    # path runs the reduction tree / transpose / matmul internals in bf16
    # (host-simulated rms rel err 4.3e-3 vs the 2e-2 gate)
    mdt = f32 if with_bias else bf16
    nc = bacc.Bacc("TRN2", target_bir_lowering=False, debug=False,
                   num_devices=NCORES)

    x = nc.declare_dram_parameter("x", [BC, N_AG, F_IN], f32, isOutput=False)
    w2 = nc.declare_dram_parameter("w2", [2, 128, F_OUT], mdt, isOutput=False)
    cvec = nc.declare_dram_parameter("cvec", [1, F_OUT], f32, isOutput=False)
    ones1 = nc.declare_dram_parameter("ones1", [1, 128], f32, isOutput=False)
    out = nc.declare_dram_parameter("out", [BC, F_OUT], f32, isOutput=True)

    xap = x.ap()
    outap = out.ap()
    relu = mybir.ActivationFunctionType.Relu

    # f32 working tiles are 2x the size; use fewer slots to stay in SBUF
    nslots = XT_SLOTS if mdt == bf16 else 5

    with tile.TileContext(nc) as tc:
        with (
            tc.tile_pool(name="sb", bufs=1) as sb,
            tc.tile_pool(name="ps", bufs=1, space="PSUM") as ps,
        ):
            w2t = sb.tile([128, 2, F_OUT], mdt)
            nc.scalar.dma_start(out=w2t[:], in_=w2.ap().rearrange("c p j -> p c j"))
            ident = sb.tile([128, 128], mdt)
            make_identity(nc, ident[:])
            if with_bias:
                ct = sb.tile([1, F_OUT], f32)
                nc.scalar.dma_start(out=ct[:], in_=cvec[:])
                o1 = sb.tile([1, 128], f32)
                nc.scalar.dma_start(out=o1[:], in_=ones1[:])

            xt = sb.tile([128, nslots, 16 * F_IN], f32)
            xb = sb.tile([128, nslots, 8 * F_IN], mdt)
            # dedicated buffers for the final small chunks: their DMA issues
            # must not wait on buffer recycling (the recycle wait rides a DVE
            # progress clock that lags the stream)
            ndedic = len(CHUNK_PLAN[-1]) - 3
            dedic = [(GROUPS - 1, c) for c in range(ndedic, ndedic + 3)]
            dag = [CHUNK_PLAN[-1][c] for _, c in dedic]
            xt3 = [sb.tile([128, ag * F_IN], f32, name=f"xt3_{i}")
                   for i, ag in enumerate(dag)]
            xb3 = [sb.tile([128, ag * F_IN // 2], mdt, name=f"xb3_{i}")
                   for i, ag in enumerate(dag)]
            ms = sb.tile([128, GROUPS, F_IN], mdt)
            mt = sb.tile([128, GROUPS, 2, GB], mdt)
            yt = sb.tile([128, GROUPS, F_OUT], f32)
            pt = ps.tile([128, 2, GB], mdt)
            py = ps.tile([128, F_OUT], f32)

            slot = 0
            dmac = 0
            for g in range(GROUPS):
                for c, ag in enumerate(CHUNK_PLAN[g]):
                    a0 = sum(CHUNK_PLAN[g][:c])
                    cols = ag * F_IN
                    if (g, c) in dedic:
                        di = dedic.index((g, c))
                        xtv, xbv = xt3[di][:], xb3[di][:]
                    else:
                        xtv = xt[:, slot, 0:cols]
                        xbv = xb[:, slot, 0:cols // 2]
                        slot = (slot + 1) % nslots
                    ieng = nc.scalar if (ALT_DMA and dmac % 2) else nc.sync
                    dmac += 1
                    ieng.dma_start(
                        out=xtv,
                        in_=xap[g * GB:(g + 1) * GB, a0:a0 + ag]
                        .rearrange("b a f -> b (a f)"))
                    # tree-reduce the chunk down to [128, 256]; the first
                    # chunk's last level writes ms directly, later chunks
                    # merge into ms
                    S = cols // 2
                    dst = (ms[:, g, :] if (c == 0 and S == F_IN)
                           else xbv[:, 0:S])
                    nc.vector.tensor_add(            # level 1: f32 -> mdt
                        dst, xtv[:, 0:S], xtv[:, S:cols])
                    while S > F_IN:
                        h = S // 2
                        dst = (ms[:, g, :] if (c == 0 and h == F_IN)
                               else xbv[:, 0:h])
                        nc.vector.tensor_add(
                            dst, xbv[:, 0:h], xbv[:, h:S])
                        S = h
                    if c > 0:
                        nc.vector.tensor_add(
                            ms[:, g, :], ms[:, g, :], xbv[:, 0:F_IN])

                for fc in range(2):
                    nc.tensor.transpose(
                        pt[:, fc, :], ms[:, g, fc * 128:(fc + 1) * 128],
                        ident[:])
                nc.scalar.copy(mt[:, g, :, :], pt[:, :, :])
                for fc in range(2):
                    nc.tensor.matmul(py[:], mt[:, g, fc, :], w2t[:, fc, :],
                                     start=(fc == 0),
                                     stop=(fc == 1 and not with_bias))
                if with_bias:
                    nc.tensor.matmul(py[:], o1[:], ct[:], start=False,
                                     stop=True)
                nc.scalar.activation(yt[:, g, :], py[:], relu)
                nc.scalar.dma_start(out=outap[g * GB:(g + 1) * GB],
                                    in_=yt[:, g, :])
    nc.compile()
    return nc


def _get_nc(with_bias):
    key = ("bias", with_bias)
    if key not in _NC_CACHE:
        _NC_CACHE[key] = _build_bass(with_bias)
    return _NC_CACHE[key]


def _is_block_pattern(node_idx, edge_idx):
    n = BATCH * N_AG * N_HE
    if node_idx.shape != (n,) or edge_idx.shape != (n,):
        return False
    i = np.arange(n, dtype=np.int64)
    if not np.array_equal(node_idx.astype(np.int64), i // N_HE):
        return False
    return np.array_equal(edge_idx.astype(np.int64),
                          (i // (N_AG * N_HE)) * N_HE + (i % N_HE))


def _fallback(inp, lin_w, hgcn_bias, out_w, out_b, node_idx, edge_idx):
    # general (host) path for arbitrary incidence — only used if the indices
    # are not the block-diagonal pattern produced by the reference setup
    n_nodes = BATCH * N_AG
    n_edges = BATCH * N_HE
    x = inp.reshape(-1, F_IN) @ lin_w
    node_idx = node_idx.astype(np.int64)
    edge_idx = edge_idx.astype(np.int64)
    D = np.bincount(node_idx, minlength=n_nodes).astype(np.float32)
    deg = np.bincount(edge_idx, minlength=n_edges).astype(np.float32)
    D_inv = np.where(D > 0, 1.0 / np.maximum(D, 1), 0.0).astype(np.float32)
    B_inv = np.where(deg > 0, 1.0 / np.maximum(deg, 1), 0.0).astype(np.float32)
    edge_feat = np.zeros((n_edges, F_OUT), np.float32)
    np.add.at(edge_feat, edge_idx, x[node_idx] * B_inv[edge_idx][:, None])
    outp = np.zeros((n_nodes, F_OUT), np.float32)
    np.add.at(outp, node_idx, edge_feat[edge_idx] * D_inv[node_idx][:, None])
    outp += hgcn_bias
    return np.maximum(outp @ out_w + out_b, 0.0)


def kernel(**inputs):
    global LAST_RESULT
    inp = np.ascontiguousarray(np.asarray(inputs["input"], np.float32))
    lin_w = np.asarray(inputs["lin_w"], np.float32)
    hgcn_bias = np.asarray(inputs["hgcn_bias"], np.float32)
    out_w = np.asarray(inputs["out_w"], np.float32)
    out_b = np.asarray(inputs["out_b"], np.float32)
    node_idx = np.asarray(inputs["node_idx"])
    edge_idx = np.asarray(inputs["edge_idx"])

    if not _is_block_pattern(node_idx, edge_idx):
        return _fallback(inp, lin_w, hgcn_bias, out_w, out_b,
                         node_idx, edge_idx)

    # fold: y = relu(mean_a(input) @ (lin_w @ out_w) + hgcn_bias @ out_w + out_b)
    w64 = lin_w.astype(np.float64) @ out_w.astype(np.float64)
    W = (w64 / N_AG).astype(np.float32)
    c = (hgcn_bias.astype(np.float64) @ out_w.astype(np.float64)
         + out_b).astype(np.float32)
    with_bias = bool(np.any(c != 0.0))

    w2 = np.ascontiguousarray(W.reshape(2, 128, F_OUT))
    if not with_bias:
        import ml_dtypes
        w2 = np.ascontiguousarray(w2.astype(ml_dtypes.bfloat16))
    cvec = np.ascontiguousarray(c.reshape(1, F_OUT))
    ones1 = np.ones((1, 128), np.float32)

    from concourse.bass_utils import run_bass_kernel_spmd

    nc = _get_nc(with_bias)
    in_maps = [
        {"x": inp[i * BC:(i + 1) * BC], "w2": w2, "cvec": cvec,
         "ones1": ones1}
        for i in range(NCORES)
    ]
    res = run_bass_kernel_spmd(nc, in_maps, list(range(NCORES)), trace=TRACE)
    LAST_RESULT = res
    y = np.concatenate([res.results[i]["out"] for i in range(NCORES)],
                       axis=0)                          # [BATCH, F_OUT]
    full = np.broadcast_to(y[:, None, :], (BATCH, N_AG, F_OUT))
    return np.ascontiguousarray(full).reshape(BATCH * N_AG, F_OUT)
